# revision 14
# baseline (speedup 1.0000x reference)
"""MCAM kernel (per-core program), v3.

Per core (one sample b):
  f_b = W_b @ x_b   (1x1 conv, fp32r matmuls, f32 PSUM) -> f16_b fp16 [c | pix]
  G   = PE-transpose of f16 (fp16, 1 cyc/row)            [(h, chalf) | k, w]
  S_c = F_c^T F_c   (fp16 gram, f32 PSUM)                [(i, chalf) | j, c-slot]
      c-slot is the INNERMOST (contiguous) free dim so per-j softmax ops
      run dense: DVE chunked max-reduce, then per-j ACT
      E = exp(S - M) with bias=-M and accum_out=Z (no separate sub/sum passes)
  had = E_o * E_s (fp16, in-place into E_o);  had2 = Square(had * rc) via ACT scale
  PE-transpose had2 back to [c | pix] (no DRAM round trip)
  att = (had2 * f16_o) * f16_s  (f32 out)

No DRAM spills at all; S + E stay in SBUF, branches processed sequentially.
"""
from contextlib import ExitStack

import numpy as np

import concourse.bass as bass
import concourse.bacc as bacc
import concourse.mybir as mybir
import concourse.tile as tile
from concourse.masks import make_identity

F32 = mybir.dt.float32
F32R = mybir.dt.float32r
F16 = mybir.dt.float16
AL = mybir.AluOpType
AF = mybir.ActivationFunctionType
AX = mybir.AxisListType

C, HH, WW = 512, 64, 64
PIX = HH * WW  # 4096
NM = 4
NK = 4
NSLAB = 8
PITCH = 64 * 256  # S free-pitch per partition: [j 64, c-slot 256]


def rap(t, dims, off=0):
    return bass.AP(tensor=t.tensor, offset=t.offset + off, ap=[list(d) for d in dims])


def build_core():
    nc = bacc.Bacc("TRN2", target_bir_lowering=False, debug=False)
    x_dram = {
        "o": nc.dram_tensor("x_opt", [C, PIX], F32R, kind="ExternalInput").ap(),
        "s": nc.dram_tensor("x_sar", [C, PIX], F32R, kind="ExternalInput").ap(),
    }
    w_dram = {
        "o": nc.dram_tensor("w_opt", [C, C], F32, kind="ExternalInput").ap(),
        "s": nc.dram_tensor("w_sar", [C, C], F32, kind="ExternalInput").ap(),
    }
    att = nc.dram_tensor("att", [C, PIX], F32, kind="ExternalOutput").ap()

    with tile.TileContext(nc) as tc, ExitStack() as ctx:
        persist = ctx.enter_context(tc.tile_pool(name="persist", bufs=1))
        smalls = ctx.enter_context(tc.tile_pool(name="smalls", bufs=1))
        cps = ctx.enter_context(tc.tile_pool(name="cps", bufs=2, space="PSUM"))
        tps = ctx.enter_context(tc.tile_pool(name="tps", bufs=2, space="PSUM"))
        gps = ctx.enter_context(tc.tile_pool(name="gps", bufs=2, space="PSUM"))

        ident = persist.tile([128, 128], F32, name="ident")
        make_identity(nc, ident)
        ident16 = persist.tile([128, 128], F16, name="ident16")
        make_identity(nc, ident16)
        f16 = {
            "o": persist.tile([128, NM, PIX], F16, name="f16_o"),
            "s": persist.tile([128, NM, PIX], F16, name="f16_s"),
        }
        had = persist.tile([128, 64, 256], F16, name="had")
        Zp = {
            "o": smalls.tile([128, 64], F32, name="Zp_o"),
            "s": smalls.tile([128, 64], F32, name="Zp_s"),
        }

        def load_wt(b, pool):
            """WT[ci_p, k, co] = W[co, k*128+ci_p]"""
            WT = pool.tile([128, NK, C], F32R, tag="WT")
            wsb = pool.tile([128, NM, C], F32, tag="wsb")
            nc.sync.dma_start(
                out=wsb, in_=w_dram[b].rearrange("(m p) ci -> p m ci", p=128)
            )
            for ko in range(NK):
                wps = cps.tile([128, C], F32, tag="cp")
                for mo in range(NM):
                    nc.tensor.transpose(
                        wps[:, mo * 128:(mo + 1) * 128],
                        in_=wsb[:, mo, ko * 128:(ko + 1) * 128],
                        identity=ident,
                    )
                nc.scalar.copy(out=WT[:, ko, :], in_=wps)
            return WT

        def conv(b, f_out, WT, pool, evac="scalar"):
            for slab in range(NSLAB):
                xt = pool.tile([128, NK, 512], F32R, tag="xt")
                for k in range(NK):
                    nc.sync.dma_start(
                        out=xt[:, k, :],
                        in_=x_dram[b][k * 128:(k + 1) * 128,
                                      slab * 512:(slab + 1) * 512],
                    )
                for m in range(NM):
                    cp = cps.tile([128, 512], F32, tag="cp")
                    for k in range(NK):
                        nc.tensor.matmul(
                            cp,
                            lhsT=WT[:, k, m * 128:(m + 1) * 128],
                            rhs=xt[:, k, :],
                            start=(k == 0),
                            stop=(k == NK - 1),
                        )
                    sl = slice(slab * 512, (slab + 1) * 512)
                    if evac == "scalar":
                        nc.scalar.copy(out=f_out[:, m, sl], in_=cp)
                    else:
                        nc.vector.tensor_copy(out=f_out[:, m, sl], in_=cp)

        def transpose_gram(b, f_raw, S, gpool):
            """S[(h2,i) | j, c-slot] with c = h2*256 + slot, slot = mpar*128+kl."""
            for mpar in range(2):
                G = gpool.tile([128, 128, WW], F16, tag="G")
                for wq in range(16):
                    for half, m in ((0, mpar), (1, mpar + 2)):
                        tp = tps.tile([64, 512], F16, tag="tp")
                        for wi in range(4):
                            w = wq * 4 + wi
                            src = rap(
                                f_raw[:, m, :], [[NM * PIX, 128], [WW, HH]], off=w
                            )
                            nc.tensor.transpose(
                                tp[:, wi * 128:(wi + 1) * 128],
                                in_=src,
                                identity=ident16,
                            )
                        # tp (64p=h, (wi 4 @128, cl 128 @1)) -> G[half, cl, wq*4+wi]
                        dst = rap(
                            G[half * 64:(half + 1) * 64, :, :],
                            [[128 * WW, 64], [WW, 128], [1, 4]],
                            off=wq * 4,
                        )
                        srcap = rap(tp, [[512, 64], [1, 128], [128, 4]])
                        nc.scalar.copy(out=dst, in_=srcap)
                for kg in range(16):
                    gp = gps.tile([128, 512], F32, tag="gp")
                    for sl in range(8):
                        kl = kg * 8 + sl
                        a0 = G[0:64, kl, :]
                        nc.tensor.matmul(
                            gp[0:64, sl * 64:(sl + 1) * 64],
                            lhsT=a0, rhs=a0, start=True, stop=True,
                        )
                        a1 = G[64:128, kl, :]
                        nc.tensor.matmul(
                            gp[64:128, sl * 64:(sl + 1) * 64],
                            lhsT=a1, rhs=a1, start=True, stop=True,
                        )
                    # gp[p, sl*64+j] -> S[p, j, k0+sl]  (c contiguous innermost)
                    k0 = mpar * 128 + kg * 8
                    nc.vector.tensor_copy(
                        out=rap(S, [[PITCH, 128], [256, 64], [1, 8]], off=k0),
                        in_=rap(gp, [[512, 128], [1, 64], [64, 8]]),
                    )

        def softmax_fused(b, S, epool, mxp):
            """E = exp(S - max_c S) per (i, j); Z sums via ACT accum_out.

            Branch o writes E into `had`; branch s multiplies into `had`.
            """
            for jc in range(4):
                j0 = jc * 16
                Mp = mxp.tile([128, 16], F32, tag="Mp")
                nc.vector.tensor_reduce(
                    out=Mp,
                    in_=rap(S, [[PITCH, 128], [256, 16], [1, 256]], off=j0 * 256),
                    axis=AX.X,
                    op=AL.max,
                )
                tmp = mxp.tile([64, 16], F32, tag="tmp")
                nc.scalar.copy(out=tmp, in_=Mp[64:128])
                nMx = mxp.tile([128, 16], F32, tag="nMx")
                nc.vector.tensor_tensor(
                    out=nMx[0:64], in0=Mp[0:64], in1=tmp, op=AL.max
                )
                nc.vector.tensor_scalar_mul(
                    out=nMx[0:64], in0=nMx[0:64], scalar1=-1.0
                )
                nc.scalar.copy(out=nMx[64:128], in_=nMx[0:64])
                for j in range(16):
                    jj = j0 + j
                    src = S[:, jj, :]
                    if b == "o":
                        nc.scalar.activation(
                            out=had[:, jj, :],
                            in_=src,
                            func=AF.Exp,
                            bias=nMx[:, j:j + 1],
                            accum_out=Zp[b][:, jj:jj + 1],
                        )
                    else:
                        eb = epool.tile([128, 256], F16, tag="eb")
                        nc.scalar.activation(
                            out=eb,
                            in_=src,
                            func=AF.Exp,
                            bias=nMx[:, j:j + 1],
                            accum_out=Zp[b][:, jj:jj + 1],
                        )
                        nc.vector.tensor_tensor(
                            out=had[:, jj, :], in0=had[:, jj, :], in1=eb,
                            op=AL.mult,
                        )

        # ================= schedule =================
        # Branch o: conv, transpose+gram, then branch-s conv is issued
        # under branch-o softmax (PE + DMA run while ACT/DVE do softmax).
        with tc.tile_pool(name="xw_o", bufs=2) as xw:
            WT = load_wt("o", xw)
            conv("o", f16["o"], WT, xw)
        with (
            tc.tile_pool(name="sg_o", bufs=1) as sg,
            tc.tile_pool(name="ep_o", bufs=2) as epool,
            tc.tile_pool(name="mx_o", bufs=2) as mxp,
        ):
            S = sg.tile([128, 64, 256], F32, tag="S")
            with tc.tile_pool(name="gpool_o", bufs=1) as gpool:
                transpose_gram("o", f16["o"], S, gpool)
            with (
                tc.tile_pool(name="w_s", bufs=1) as wp,
                tc.tile_pool(name="x_s", bufs=2) as xp,
            ):
                WT_s = load_wt("s", wp)
                softmax_fused("o", S, epool, mxp)
                conv("s", f16["s"], WT_s, xp, evac="vector")
        with (
            tc.tile_pool(name="sg_s", bufs=1) as sg,
            tc.tile_pool(name="gpool_s", bufs=1) as gpool,
            tc.tile_pool(name="ep_s", bufs=2) as epool,
            tc.tile_pool(name="mx_s", bufs=2) as mxp,
        ):
            S = sg.tile([128, 64, 256], F32, tag="S")
            transpose_gram("s", f16["s"], S, gpool)
            softmax_fused("s", S, epool, mxp)

        # 1/(Z_o * Z_s), folded into Square via ACT scale
        Zt = {}
        for b in "os":
            t2 = smalls.tile([64, 64], F32, name=f"zt2_{b}")
            nc.scalar.copy(out=t2, in_=Zp[b][64:128])
            Zt[b] = smalls.tile([64, 64], F32, name=f"zt_{b}")
            nc.vector.tensor_tensor(out=Zt[b], in0=Zp[b][0:64], in1=t2, op=AL.add)
        zz = smalls.tile([64, 64], F32, name="zz")
        nc.vector.tensor_tensor(out=zz, in0=Zt["o"], in1=Zt["s"], op=AL.mult)
        rc2 = smalls.tile([128, 64], F32, name="rc2")
        nc.vector.reciprocal(out=rc2[0:64], in_=zz)
        nc.scalar.copy(out=rc2[64:128], in_=rc2[0:64])

        # had2 = (had * rc)^2, in place, per-j so rc is a per-partition scalar
        for jj in range(64):
            nc.scalar.activation(
                out=had[:, jj, :], in_=had[:, jj, :], func=AF.Square,
                scale=rc2[:, jj:jj + 1],
            )

        # PE-transpose had2 [(h2,i) | j, slot] -> hc [c | i, j] and combine:
        # att = (had2 * f_o) * f_s.  c = h2*256 + sb*128 + p  => m = h2*2 + sb.
        with (
            tc.tile_pool(name="hcp", bufs=2) as hcp,
            tc.tile_pool(name="apool", bufs=2) as apool,
            tc.tile_pool(name="tops", bufs=2, space="PSUM") as tops,
        ):
            for m in range(NM):
                h2, sb = m // 2, m % 2
                hc = hcp.tile([128, PIX], F16, tag="hc")
                for jg in range(8):
                    tpo = tops.tile([128, 8, 64], F16, tag="tpo")
                    for j8 in range(8):
                        j = jg * 8 + j8
                        nc.tensor.transpose(
                            tpo[:, j8, :],
                            in_=had[h2 * 64:(h2 + 1) * 64, j,
                                    sb * 128:(sb + 1) * 128],
                            identity=ident16[h2 * 64:(h2 + 1) * 64,
                                             h2 * 64:(h2 + 1) * 64],
                        )
                    # tpo[p, j8, i] -> hc[p, i*64 + jg*8 + j8]
                    dst = rap(hc, [[PIX, 128], [1, 8], [64, 64]], off=jg * 8)
                    srcap = rap(tpo, [[512, 128], [64, 8], [1, 64]])
                    nc.vector.tensor_copy(out=dst, in_=srcap)
                for ih in range(2):
                    psl = slice(ih * 2048, (ih + 1) * 2048)
                    vv = apool.tile([128, PIX // 2], F16, tag="vv")
                    nc.vector.tensor_tensor(
                        out=vv, in0=hc[:, psl], in1=f16["o"][:, m, psl], op=AL.mult
                    )
                    at = apool.tile([128, PIX // 2], F32, tag="at")
                    nc.vector.tensor_tensor(
                        out=at, in0=vv, in1=f16["s"][:, m, psl], op=AL.mult
                    )
                    nc.sync.dma_start(
                        out=att[m * 128:(m + 1) * 128, psl], in_=at
                    )

    nc.compile()
    return nc


_NC_CACHE = {}


def _get_nc():
    if "nc" not in _NC_CACHE:
        _NC_CACHE["nc"] = build_core()
    return _NC_CACHE["nc"]


def kernel(opt, sar, W_opt, W_sar):
    """Full inputs (8,512,64,64)x2 + (512,512)x2 -> full output (8,512,64,64).

    Data-parallel over batch: one sample per NeuronCore.
    """
    from concourse.bass_utils import run_bass_kernel_spmd

    B = opt.shape[0]
    nc = _get_nc()
    in_maps = [
        {
            "x_opt": np.ascontiguousarray(opt[b].reshape(C, PIX), dtype=np.float32),
            "x_sar": np.ascontiguousarray(sar[b].reshape(C, PIX), dtype=np.float32),
            "w_opt": np.ascontiguousarray(W_opt, dtype=np.float32),
            "w_sar": np.ascontiguousarray(W_sar, dtype=np.float32),
        }
        for b in range(B)
    ]
    res = run_bass_kernel_spmd(nc, in_maps, core_ids=list(range(B)))
    out = np.stack([res.results[b]["att"].reshape(C, HH, WW) for b in range(B)])
    return out.astype(np.float32)


# revision 16
# speedup vs baseline: 1.0892x; 1.0892x over previous
"""MCAM kernel (per-core program), v3.

Per core (one sample b):
  f_b = W_b @ x_b   (1x1 conv, fp32r matmuls, f32 PSUM) -> f16_b fp16 [c | pix]
  G   = PE-transpose of f16 (fp16, 1 cyc/row)            [(h, chalf) | k, w]
  S_c = F_c^T F_c   (fp16 gram, f32 PSUM)                [(i, chalf) | j, c-slot]
      c-slot is the INNERMOST (contiguous) free dim so per-j softmax ops
      run dense: DVE chunked max-reduce, then per-j ACT
      E = exp(S - M) with bias=-M and accum_out=Z (no separate sub/sum passes)
  had = E_o * E_s (fp16, in-place into E_o);  had2 = Square(had * rc) via ACT scale
  PE-transpose had2 back to [c | pix] (no DRAM round trip)
  att = (had2 * f16_o) * f16_s  (f32 out)

No DRAM spills at all; S + E stay in SBUF, branches processed sequentially.
"""
from contextlib import ExitStack

import numpy as np

import concourse.bass as bass
import concourse.bacc as bacc
import concourse.mybir as mybir
import concourse.tile as tile
from concourse.masks import make_identity

F32 = mybir.dt.float32
F32R = mybir.dt.float32r
F16 = mybir.dt.float16
AL = mybir.AluOpType
AF = mybir.ActivationFunctionType
AX = mybir.AxisListType

C, HH, WW = 512, 64, 64
PIX = HH * WW  # 4096
NM = 4
NK = 4
NSLAB = 8
PITCH = 64 * 256  # S free-pitch per partition: [j 64, c-slot 256]


def rap(t, dims, off=0):
    return bass.AP(tensor=t.tensor, offset=t.offset + off, ap=[list(d) for d in dims])


def build_core():
    nc = bacc.Bacc("TRN2", target_bir_lowering=False, debug=False)
    x_dram = {
        "o": nc.dram_tensor("x_opt", [C, PIX], F32R, kind="ExternalInput").ap(),
        "s": nc.dram_tensor("x_sar", [C, PIX], F32R, kind="ExternalInput").ap(),
    }
    w_dram = {
        "o": nc.dram_tensor("w_opt", [C, C], F32, kind="ExternalInput").ap(),
        "s": nc.dram_tensor("w_sar", [C, C], F32, kind="ExternalInput").ap(),
    }
    att = nc.dram_tensor("att", [C, PIX], F32, kind="ExternalOutput").ap()

    with tile.TileContext(nc) as tc, ExitStack() as ctx:
        persist = ctx.enter_context(tc.tile_pool(name="persist", bufs=1))
        smalls = ctx.enter_context(tc.tile_pool(name="smalls", bufs=1))
        cps = ctx.enter_context(tc.tile_pool(name="cps", bufs=2, space="PSUM"))
        tps = ctx.enter_context(tc.tile_pool(name="tps", bufs=2, space="PSUM"))
        gps = ctx.enter_context(tc.tile_pool(name="gps", bufs=2, space="PSUM"))

        ident = persist.tile([128, 128], F32, name="ident")
        make_identity(nc, ident)
        ident16 = persist.tile([128, 128], F16, name="ident16")
        make_identity(nc, ident16)
        f16 = {
            "o": persist.tile([128, NM, PIX], F16, name="f16_o"),
            "s": persist.tile([128, NM, PIX], F16, name="f16_s"),
        }
        had = persist.tile([128, 64, 256], F16, name="had")
        Zp = {
            "o": smalls.tile([128, 64], F32, name="Zp_o"),
            "s": smalls.tile([128, 64], F32, name="Zp_s"),
        }

        def load_wt(b, pool):
            """WT[ci_p, k, co] = W[co, k*128+ci_p]"""
            WT = pool.tile([128, NK, C], F32R, tag="WT")
            wsb = pool.tile([128, NM, C], F32, tag="wsb")
            nc.sync.dma_start(
                out=wsb, in_=w_dram[b].rearrange("(m p) ci -> p m ci", p=128)
            )
            for ko in range(NK):
                wps = cps.tile([128, C], F32, tag="cp")
                for mo in range(NM):
                    nc.tensor.transpose(
                        wps[:, mo * 128:(mo + 1) * 128],
                        in_=wsb[:, mo, ko * 128:(ko + 1) * 128],
                        identity=ident,
                    )
                nc.scalar.copy(out=WT[:, ko, :], in_=wps)
            return WT

        def conv(b, f_out, WT, pool, evac="scalar"):
            for slab in range(NSLAB):
                xt = pool.tile([128, NK, 512], F32R, tag="xt")
                for k in range(NK):
                    nc.sync.dma_start(
                        out=xt[:, k, :],
                        in_=x_dram[b][k * 128:(k + 1) * 128,
                                      slab * 512:(slab + 1) * 512],
                    )
                for m in range(NM):
                    cp = cps.tile([128, 512], F32, tag="cp")
                    for k in range(NK):
                        nc.tensor.matmul(
                            cp,
                            lhsT=WT[:, k, m * 128:(m + 1) * 128],
                            rhs=xt[:, k, :],
                            start=(k == 0),
                            stop=(k == NK - 1),
                        )
                    sl = slice(slab * 512, (slab + 1) * 512)
                    if evac == "scalar":
                        nc.scalar.copy(out=f_out[:, m, sl], in_=cp)
                    else:
                        nc.vector.tensor_copy(out=f_out[:, m, sl], in_=cp)

        def transpose_gram(b, f_raw, S, gpool):
            """S[(h2,i) | j, c-slot] with c = h2*256 + slot, slot = mpar*128+kl."""
            for mpar in range(2):
                G = gpool.tile([128, 128, WW], F16, tag="G")
                for wq in range(16):
                    for half, m in ((0, mpar), (1, mpar + 2)):
                        tp = tps.tile([64, 512], F16, tag="tp")
                        for wi in range(4):
                            w = wq * 4 + wi
                            src = rap(
                                f_raw[:, m, :], [[NM * PIX, 128], [WW, HH]], off=w
                            )
                            nc.tensor.transpose(
                                tp[:, wi * 128:(wi + 1) * 128],
                                in_=src,
                                identity=ident16,
                            )
                        # tp (64p=h, (wi 4 @128, cl 128 @1)) -> G[half, cl, wq*4+wi]
                        dst = rap(
                            G[half * 64:(half + 1) * 64, :, :],
                            [[128 * WW, 64], [WW, 128], [1, 4]],
                            off=wq * 4,
                        )
                        srcap = rap(tp, [[512, 64], [1, 128], [128, 4]])
                        nc.scalar.copy(out=dst, in_=srcap)
                for kg in range(16):
                    gp = gps.tile([128, 512], F32, tag="gp")
                    for sl in range(8):
                        kl = kg * 8 + sl
                        a0 = G[0:64, kl, :]
                        nc.tensor.matmul(
                            gp[0:64, sl * 64:(sl + 1) * 64],
                            lhsT=a0, rhs=a0, start=True, stop=True,
                        )
                        a1 = G[64:128, kl, :]
                        nc.tensor.matmul(
                            gp[64:128, sl * 64:(sl + 1) * 64],
                            lhsT=a1, rhs=a1, start=True, stop=True,
                        )
                    # gp[p, sl*64+j] -> S[p, j, k0+sl]  (c contiguous innermost)
                    k0 = mpar * 128 + kg * 8
                    nc.vector.tensor_copy(
                        out=rap(S, [[PITCH, 128], [256, 64], [1, 8]], off=k0),
                        in_=rap(gp, [[512, 128], [1, 64], [64, 8]]),
                    )

        def softmax_fused(b, S, epool, mxp):
            """E = exp(S - max_c S) per (i, j); Z sums via ACT accum_out.

            Branch o writes E into `had`; branch s multiplies into `had`.
            """
            for jc in range(4):
                j0 = jc * 16
                Mp = mxp.tile([128, 16], F32, tag="Mp")
                nc.vector.tensor_reduce(
                    out=Mp,
                    in_=rap(S, [[PITCH, 128], [256, 16], [1, 256]], off=j0 * 256),
                    axis=AX.X,
                    op=AL.max,
                )
                tmp = mxp.tile([64, 16], F32, tag="tmp")
                nc.scalar.copy(out=tmp, in_=Mp[64:128])
                nMx = mxp.tile([128, 16], F32, tag="nMx")
                nc.vector.tensor_tensor(
                    out=nMx[0:64], in0=Mp[0:64], in1=tmp, op=AL.max
                )
                nc.vector.tensor_scalar_mul(
                    out=nMx[0:64], in0=nMx[0:64], scalar1=-1.0
                )
                nc.scalar.copy(out=nMx[64:128], in_=nMx[0:64])
                for j in range(16):
                    jj = j0 + j
                    src = S[:, jj, :]
                    if b == "o":
                        nc.scalar.activation(
                            out=had[:, jj, :],
                            in_=src,
                            func=AF.Exp,
                            bias=nMx[:, j:j + 1],
                            accum_out=Zp[b][:, jj:jj + 1],
                        )
                    else:
                        eb = epool.tile([128, 256], F16, tag="eb")
                        nc.scalar.activation(
                            out=eb,
                            in_=src,
                            func=AF.Exp,
                            bias=nMx[:, j:j + 1],
                            accum_out=Zp[b][:, jj:jj + 1],
                        )
                        nc.vector.tensor_tensor(
                            out=had[:, jj, :], in0=had[:, jj, :], in1=eb,
                            op=AL.mult,
                        )

        # ================= schedule =================
        for b in "os":
            with tc.tile_pool(name=f"xw_{b}", bufs=2) as xw:
                WT = load_wt(b, xw)
                conv(b, f16[b], WT, xw)
            with (
                tc.tile_pool(name=f"sg_{b}", bufs=1) as sg,
                tc.tile_pool(name=f"gpool_{b}", bufs=1) as gpool,
                tc.tile_pool(name=f"ep_{b}", bufs=2) as epool,
                tc.tile_pool(name=f"mx_{b}", bufs=2) as mxp,
            ):
                S = sg.tile([128, 64, 256], F32, tag="S")
                transpose_gram(b, f16[b], S, gpool)
                softmax_fused(b, S, epool, mxp)

        # 1/(Z_o * Z_s), folded into Square via ACT scale
        Zt = {}
        for b in "os":
            t2 = smalls.tile([64, 64], F32, name=f"zt2_{b}")
            nc.scalar.copy(out=t2, in_=Zp[b][64:128])
            Zt[b] = smalls.tile([64, 64], F32, name=f"zt_{b}")
            nc.vector.tensor_tensor(out=Zt[b], in0=Zp[b][0:64], in1=t2, op=AL.add)
        zz = smalls.tile([64, 64], F32, name="zz")
        nc.vector.tensor_tensor(out=zz, in0=Zt["o"], in1=Zt["s"], op=AL.mult)
        rc2 = smalls.tile([128, 64], F32, name="rc2")
        nc.vector.reciprocal(out=rc2[0:64], in_=zz)
        nc.scalar.copy(out=rc2[64:128], in_=rc2[0:64])

        # had2 = (had * rc)^2, in place, per-j so rc is a per-partition scalar
        for jj in range(64):
            nc.scalar.activation(
                out=had[:, jj, :], in_=had[:, jj, :], func=AF.Square,
                scale=rc2[:, jj:jj + 1],
            )

        # PE-transpose had2 [(h2,i) | j, slot] -> hc [c | i, j] and combine:
        # att = (had2 * f_o) * f_s.  c = h2*256 + sb*128 + p  => m = h2*2 + sb.
        with (
            tc.tile_pool(name="hcp", bufs=2) as hcp,
            tc.tile_pool(name="apool", bufs=2) as apool,
            tc.tile_pool(name="tops", bufs=2, space="PSUM") as tops,
        ):
            for m in range(NM):
                h2, sb = m // 2, m % 2
                hc = hcp.tile([128, PIX], F16, tag="hc")
                for jg in range(8):
                    tpo = tops.tile([128, 8, 64], F16, tag="tpo")
                    for j8 in range(8):
                        j = jg * 8 + j8
                        nc.tensor.transpose(
                            tpo[:, j8, :],
                            in_=had[h2 * 64:(h2 + 1) * 64, j,
                                    sb * 128:(sb + 1) * 128],
                            identity=ident16[h2 * 64:(h2 + 1) * 64,
                                             h2 * 64:(h2 + 1) * 64],
                        )
                    # tpo[p, j8, i] -> hc[p, i*64 + jg*8 + j8]
                    dst = rap(hc, [[PIX, 128], [1, 8], [64, 64]], off=jg * 8)
                    srcap = rap(tpo, [[512, 128], [64, 8], [1, 64]])
                    if jg % 2 == 0:
                        nc.scalar.copy(out=dst, in_=srcap)
                    else:
                        nc.vector.tensor_copy(out=dst, in_=srcap)
                for ih in range(2):
                    psl = slice(ih * 2048, (ih + 1) * 2048)
                    vv = apool.tile([128, PIX // 2], F16, tag="vv")
                    nc.vector.tensor_tensor(
                        out=vv, in0=hc[:, psl], in1=f16["o"][:, m, psl], op=AL.mult
                    )
                    at = apool.tile([128, PIX // 2], F32, tag="at")
                    nc.vector.tensor_tensor(
                        out=at, in0=vv, in1=f16["s"][:, m, psl], op=AL.mult
                    )
                    nc.sync.dma_start(
                        out=att[m * 128:(m + 1) * 128, psl], in_=at
                    )

    nc.compile()
    return nc


_NC_CACHE = {}


def _get_nc():
    if "nc" not in _NC_CACHE:
        _NC_CACHE["nc"] = build_core()
    return _NC_CACHE["nc"]


def kernel(opt, sar, W_opt, W_sar):
    """Full inputs (8,512,64,64)x2 + (512,512)x2 -> full output (8,512,64,64).

    Data-parallel over batch: one sample per NeuronCore.
    """
    from concourse.bass_utils import run_bass_kernel_spmd

    B = opt.shape[0]
    nc = _get_nc()
    in_maps = [
        {
            "x_opt": np.ascontiguousarray(opt[b].reshape(C, PIX), dtype=np.float32),
            "x_sar": np.ascontiguousarray(sar[b].reshape(C, PIX), dtype=np.float32),
            "w_opt": np.ascontiguousarray(W_opt, dtype=np.float32),
            "w_sar": np.ascontiguousarray(W_sar, dtype=np.float32),
        }
        for b in range(B)
    ]
    res = run_bass_kernel_spmd(nc, in_maps, core_ids=list(range(B)))
    out = np.stack([res.results[b]["att"].reshape(C, HH, WW) for b in range(B)])
    return out.astype(np.float32)
